# revision 1
# baseline (speedup 1.0000x reference)
"""Bidirectional ConvLSTM Trainium2 kernel (8-core SPMD).

Sharding: 8 sequences = 4 batches x 2 directions; core 2b = forward for
batch b, core 2b+1 = backward (host feeds time-reversed x and that
direction's weights). No cross-core traffic during the T-loop; fusion via
one pairwise AllGather of the per-step 1x1-conv partials, BatchNorm via an
8-core AllReduce of channel stats.
"""

import numpy as np
import concourse.bass as bass
import concourse.bacc as bacc
import concourse.mybir as mybir
import concourse.tile as tile
from concourse.bass_utils import run_bass_kernel_spmd

fp32 = mybir.dt.float32
fp32r = mybir.dt.float32r
i32 = mybir.dt.int32
Alu = mybir.AluOpType
Act = mybir.ActivationFunctionType

T = 16
HID = 64
S = 4096  # 64*64 spatial
EPS = 1e-5
N_CORES = 8
CORE_IDS = list(range(N_CORES))
MAGIC = 0x5F3759DF


def emit_rsqrt(nc, pool, x_ap, out_ap, iters=3):
    """out = 1/sqrt(x) via bit-trick seed + Newton, DVE only. x_ap fp32 [P,1]."""
    P = x_ap.shape[0]
    yi = pool.tile([P, 1], i32, tag=f"rsq_i{P}")
    t = pool.tile([P, 1], fp32, tag=f"rsq_t{P}")
    nc.vector.tensor_scalar(yi[:], x_ap.bitcast(i32), 1, None,
                            Alu.logical_shift_right)
    nc.vector.tensor_scalar(yi[:], yi[:], -1, MAGIC, Alu.mult, Alu.add)
    y = yi[:].bitcast(fp32)
    for i in range(iters):
        nc.vector.tensor_mul(t[:], y, y)
        nc.vector.tensor_mul(t[:], t[:], x_ap)
        nc.vector.tensor_scalar(t[:], t[:], -0.5, 1.5, Alu.mult, Alu.add)
        nc.vector.tensor_mul(out_ap if i == iters - 1 else y, y, t[:])


def build_program(nsteps=T, reps=1, chunk=2048):
    nc = bacc.Bacc("TRN2", target_bir_lowering=False, debug=False,
                   num_devices=N_CORES)

    xs = nc.dram_tensor("xs", [nsteps, 64, S], fp32, kind="ExternalInput").ap()
    wconv = nc.dram_tensor("wconv", [9, 2, 128, 128], fp32, kind="ExternalInput").ap()
    wfu = nc.dram_tensor("wfu", [128, 64], fp32, kind="ExternalInput").ap()
    gnw = nc.dram_tensor("gnw", [2, 128], fp32, kind="ExternalInput").ap()
    gnb = nc.dram_tensor("gnb", [2, 128], fp32, kind="ExternalInput").ap()
    bnw = nc.dram_tensor("bnw", [64, 1], fp32, kind="ExternalInput").ap()
    bnb = nc.dram_tensor("bnb", [64, 1], fp32, kind="ExternalInput").ap()
    ind = nc.dram_tensor("ind", [128, 2], fp32, kind="ExternalInput").ap()
    indT = nc.dram_tensor("indT", [2, 128], fp32, kind="ExternalInput").ap()
    bind = nc.dram_tensor("bind", [128, 64], fp32, kind="ExternalInput").ap()
    bindT = nc.dram_tensor("bindT", [64, 128], fp32, kind="ExternalInput").ap()
    out = nc.dram_tensor("out", [nsteps, 64, S], fp32, kind="ExternalOutput").ap()

    psend = nc.dram_tensor("psend", [nsteps, 64, S], fp32)
    pgath = nc.dram_tensor("pgath", [2, nsteps, 64, S], fp32)
    bnps = nc.dram_tensor("bnps", [64, 2], fp32)
    bnpr = nc.dram_tensor("bnpr", [64, 2], fp32, addr_space="Shared")

    with tile.TileContext(nc) as tc:
      with tc.tile_pool(name="const", bufs=1) as cp:
        # constants used by the fusion tail — must outlive the loop pools
        bind_r = cp.tile([128, 64], fp32, tag="bindr")
        nc.sync.dma_start(bind_r[:], bind)
        bindT_r = cp.tile([64, 128], fp32, tag="bindTr")
        nc.sync.dma_start(bindT_r[:], bindT)
        bnw_sb = cp.tile([64, 1], fp32, tag="bnw")
        nc.sync.dma_start(bnw_sb[:], bnw)
        bnb_sb = cp.tile([64, 1], fp32, tag="bnb")
        nc.sync.dma_start(bnb_sb[:], bnb)

        for rep in range(reps):
          with (
            tc.tile_pool(name=f"persist{rep}", bufs=1) as pp,
            tc.tile_pool(name=f"work{rep}", bufs=2) as wp,
            tc.tile_pool(name=f"pg{rep}", bufs=3, space="PSUM") as pgpool,
            tc.tile_pool(name=f"sm{rep}", bufs=2, space="PSUM") as smpool,
          ):
            # ---- one-time prologue ----
            wr_st = pp.tile([128, 18 * 128], fp32, tag="wst")
            nc.sync.dma_start(
            wr_st[:].rearrange("k (t h m) -> k t h m", t=9, h=2),
            wconv.rearrange("t h k m -> k t h m"),
        )
            wr = pp.tile([128, 18 * 128], fp32r, tag="wr")
            nc.vector.tensor_copy(wr[:], wr_st[:])

            wfu_st = pp.tile([128, 64], fp32, tag="wfust")
            nc.sync.dma_start(wfu_st[:], wfu)
            wfu_r = pp.tile([128, 64], fp32r, tag="wfur")
            nc.vector.tensor_copy(wfu_r[:], wfu_st[:])

            ind_r = pp.tile([128, 2], fp32, tag="indr")
            nc.sync.dma_start(ind_r[:], ind)
            indT_r = pp.tile([2, 128], fp32, tag="indTr")
            nc.sync.dma_start(indT_r[:], indT)

            gnw_sb = pp.tile([128, 2], fp32, tag="gnw")
            nc.sync.dma_start(gnw_sb[:], gnw.rearrange("h p -> p h"))
            gnb_sb = pp.tile([128, 2], fp32, tag="gnb")
            nc.sync.dma_start(gnb_sb[:], gnb.rearrange("h p -> p h"))

            # persistent state
            inp0 = pp.tile([128, 66, 66], fp32r, tag="inp0")
            inp1 = pp.tile([128, 66, 66], fp32r, tag="inp1")
            nc.vector.memset(inp0[:].bitcast(fp32), 0.0)
            nc.vector.memset(inp1[:].bitcast(fp32), 0.0)
            inps = [inp0, inp1]
            if_sb = pp.tile([128, S], fp32, tag="ifsb")   # i(0:64), f(64:128)
            og_sb = pp.tile([128, S], fp32, tag="ogsb")   # g(0:64), o(64:128)
            state = pp.tile([128, S], fp32, tag="state")  # c on 64:128
            scr = pp.tile([128, S], fp32, tag="scr")      # t1/tanh_c on 64:128
            nc.vector.memset(state[64:128, :], 0.0)

            # x(0) load
            xst = wp.tile([64, S], fp32, tag="xst")
            nc.sync.dma_start(xst[:], xs[0])
            nc.vector.tensor_copy(inp0[0:64, 1:65, 1:65],
                                  xst[:].rearrange("p (a b) -> p a b", a=64))

            for t in range(nsteps):
                cur = inps[t % 2]
                nxt = inps[(t + 1) % 2]
                svs, bvs = [], []
                for half in range(2):
                    raw = if_sb if half == 0 else og_sb
                    st_t = wp.tile([128, 8, 6], fp32, tag=f"st{half}")
                    for qc in range(4):
                        pg = pgpool.tile([128, 1024], fp32, tag="pg")
                        for tap in range(9):
                            dy, dx = tap // 3, tap % 3
                            lhsT = wr[:, (tap * 2 + half) * 128:(tap * 2 + half + 1) * 128]
                            for b in range(2):
                                y0 = qc * 16 + b * 8
                                nc.tensor.matmul(
                                    pg[:, b * 512:(b + 1) * 512],
                                    lhsT,
                                    cur[:, y0 + dy:y0 + dy + 8, dx:dx + 64],
                                    start=(tap == 0), stop=(tap == 8),
                                )
                        nc.scalar.copy(raw[:, qc * 1024:(qc + 1) * 1024], pg[:])
                        for b in range(2):
                            nc.vector.bn_stats(
                                st_t[:, qc * 2 + b, :],
                                raw[:, qc * 1024 + b * 512:qc * 1024 + (b + 1) * 512])

                    # group-norm stats chain for this half
                    aggr = wp.tile([128, 2], fp32, tag=f"aggr{half}")
                    nc.vector.bn_aggr(aggr[:], st_t[:])
                    s3 = wp.tile([128, 3], fp32, tag=f"s3{half}")
                    nc.vector.tensor_copy(s3[:, 0:2], aggr[:])
                    nc.vector.tensor_mul(s3[:, 2:3], aggr[:, 0:1], aggr[:, 0:1])
                    smg = smpool.tile([2, 3], fp32, tag="sm")
                    nc.tensor.matmul(smg[:], ind_r[:], s3[:], start=True, stop=True)
                    gsb = wp.tile([2, 3], fp32, tag=f"gsb{half}")
                    nc.vector.tensor_scalar_mul(gsb[:], smg[:], 1.0 / 64.0)
                    mu2 = wp.tile([2, 1], fp32, tag=f"mu2{half}")
                    nc.vector.tensor_mul(mu2[:], gsb[:, 0:1], gsb[:, 0:1])
                    varx = wp.tile([2, 1], fp32, tag=f"varx{half}")
                    nc.vector.tensor_add(varx[:], gsb[:, 1:2], gsb[:, 2:3])
                    nc.vector.scalar_tensor_tensor(varx[:], varx[:], EPS, mu2[:],
                                                   Alu.add, Alu.subtract)
                    rstd = wp.tile([2, 1], fp32, tag=f"rstd{half}")
                    emit_rsqrt(nc, wp, varx[:], rstd[:])
                    brhs = wp.tile([2, 2], fp32, tag=f"brhs{half}")
                    nc.vector.tensor_copy(brhs[:, 0:1], rstd[:])
                    nc.vector.tensor_copy(brhs[:, 1:2], gsb[:, 0:1])
                    smb = smpool.tile([128, 2], fp32, tag="sm")
                    nc.tensor.matmul(smb[:], indT_r[:], brhs[:], start=True, stop=True)
                    sv = wp.tile([128, 1], fp32, tag=f"sv{half}")
                    nc.vector.tensor_mul(sv[:], smb[:, 0:1], gnw_sb[:, half:half + 1])
                    tv = wp.tile([128, 1], fp32, tag=f"tv{half}")
                    nc.vector.tensor_mul(tv[:], smb[:, 1:2], sv[:])
                    bv = wp.tile([128, 1], fp32, tag=f"bv{half}")
                    nc.vector.tensor_sub(bv[:], gnb_sb[:, half:half + 1], tv[:])
                    svs.append(sv)
                    bvs.append(bv)

                # normalize+activate and c/h update, chunked
                for ch in range(S // chunk):
                    sl = slice(ch * chunk, (ch + 1) * chunk)
                    nc.scalar.activation(if_sb[:, sl], if_sb[:, sl], Act.Sigmoid,
                                         bias=bvs[0][:], scale=svs[0][:])
                    nc.scalar.activation(og_sb[0:64, sl], og_sb[0:64, sl], Act.Tanh,
                                         bias=bvs[1][0:64, :], scale=svs[1][0:64, :])
                    nc.scalar.activation(og_sb[64:128, sl], og_sb[64:128, sl],
                                         Act.Sigmoid,
                                         bias=bvs[1][64:128, :], scale=svs[1][64:128, :])
                    nc.vector.tensor_mul(scr[64:128, sl], if_sb[0:64, sl],
                                         og_sb[0:64, sl])
                    nc.vector.tensor_mul(state[64:128, sl], if_sb[64:128, sl],
                                         state[64:128, sl])
                    nc.vector.tensor_add(state[64:128, sl], state[64:128, sl],
                                         scr[64:128, sl])
                    nc.scalar.activation(scr[64:128, sl], state[64:128, sl], Act.Tanh)
                    r0 = ch * (chunk // 64)
                    nrows = chunk // 64
                    nc.vector.tensor_mul(
                        nxt[64:128, 1 + r0:1 + r0 + nrows, 1:65],
                        og_sb[64:128, sl].rearrange("p (a b) -> p a b", a=nrows),
                        scr[64:128, sl].rearrange("p (a b) -> p a b", a=nrows),
                    )

                if t < nsteps - 1:
                    xst = wp.tile([64, S], fp32, tag="xst")
                    nc.sync.dma_start(xst[:], xs[t + 1])
                    nc.gpsimd.tensor_copy(nxt[0:64, 1:65, 1:65],
                                          xst[:].rearrange("p (a b) -> p a b", a=64))

                # fusion partial p_t = wfu^T @ [*, h_t]
                for k in range(8):
                    pf = smpool.tile([64, 512], fp32, tag="sm")
                    nc.tensor.matmul(pf[:], wfu_r[:],
                                     nxt[:, 1 + k * 8:1 + k * 8 + 8, 1:65],
                                     start=True, stop=True)
                    psb = wp.tile([64, 512], fp32, tag="psb")
                    if k % 2 == 0:
                        nc.scalar.copy(psb[:], pf[:])
                    else:
                        nc.vector.tensor_copy(psb[:], pf[:])
                    nc.sync.dma_start(psend[t, :, k * 512:(k + 1) * 512], psb[:])

          # ---- fusion tail ----
          nc.gpsimd.collective_compute(
            "AllGather", Alu.bypass,
            replica_groups=[[0, 1], [2, 3], [4, 5], [6, 7]],
            ins=[psend[:]], outs=[pgath[:]],
          )

          nu = nsteps // 2
          with (
            tc.tile_pool(name=f"tailp{rep}", bufs=1) as tp,
            tc.tile_pool(name=f"tailw{rep}", bufs=2) as tw,
            tc.tile_pool(name=f"tsm{rep}", bufs=2, space="PSUM") as tsm,
          ):
            F = tp.tile([128, nu, S], fp32, tag="F")
            st_t = tp.tile([128, 8 * nu, 6], fp32, tag="stT")
            for u in range(nu):
                for cc in range(2):
                    cols = slice(cc * 2048, (cc + 1) * 2048)
                    tA = tw.tile([128, 2048], fp32, tag="tA")
                    tB = tw.tile([128, 2048], fp32, tag="tB")
                    nc.sync.dma_start(tA[0:64, :], pgath[0, 2 * u, :, cols])
                    nc.sync.dma_start(tA[64:128, :], pgath[0, 2 * u + 1, :, cols])
                    nc.sync.dma_start(tB[0:64, :], pgath[1, nsteps - 1 - 2 * u, :, cols])
                    nc.sync.dma_start(tB[64:128, :], pgath[1, nsteps - 2 - 2 * u, :, cols])
                    nc.vector.tensor_add(F[:, u, cols], tA[:], tB[:])
                    for q in range(4):
                        fs = slice(cc * 2048 + q * 512, cc * 2048 + (q + 1) * 512)
                        nc.vector.bn_stats(st_t[:, u * 8 + cc * 4 + q, :],
                                           F[:, u, fs])

            aggr = tw.tile([128, 2], fp32, tag="taggr")
            nc.vector.bn_aggr(aggr[:], st_t[:])
            s2 = tw.tile([128, 2], fp32, tag="ts2")
            nc.vector.tensor_copy(s2[:, 0:1], aggr[:, 0:1])
            t128 = tw.tile([128, 1], fp32, tag="t128")
            nc.vector.tensor_mul(t128[:], aggr[:, 0:1], aggr[:, 0:1])
            nc.vector.tensor_add(s2[:, 1:2], aggr[:, 1:2], t128[:])
            smg = tsm.tile([64, 2], fp32, tag="tsm")
            nc.tensor.matmul(smg[:], bind_r[:], s2[:], start=True, stop=True)
            bsb = tw.tile([64, 2], fp32, tag="bsb")
            nc.scalar.copy(bsb[:], smg[:])
            nc.sync.dma_start(bnps[:], bsb[:])
            nc.gpsimd.collective_compute(
                "AllReduce", Alu.add,
                replica_groups=[CORE_IDS],
                ins=[bnps[:]], outs=[bnpr[:]],
            )
            s16 = tw.tile([64, 2], fp32, tag="s16")
            nc.sync.dma_start(s16[:], bnpr[:])
            mE = tw.tile([64, 2], fp32, tag="mE")
            nc.vector.tensor_scalar_mul(mE[:], s16[:], 1.0 / 16.0)
            mu2 = tw.tile([64, 1], fp32, tag="tmu2")
            nc.vector.tensor_mul(mu2[:], mE[:, 0:1], mE[:, 0:1])
            varx = tw.tile([64, 1], fp32, tag="tvarx")
            nc.vector.scalar_tensor_tensor(varx[:], mE[:, 1:2], EPS, mu2[:],
                                           Alu.add, Alu.subtract)
            rstd = tw.tile([64, 1], fp32, tag="trstd")
            emit_rsqrt(nc, tw, varx[:], rstd[:])
            brhs = tw.tile([64, 2], fp32, tag="tbrhs")
            nc.vector.tensor_mul(brhs[:, 0:1], bnw_sb[:], rstd[:])
            tv = tw.tile([64, 1], fp32, tag="ttv")
            nc.vector.tensor_mul(tv[:], mE[:, 0:1], brhs[:, 0:1])
            nc.vector.tensor_sub(brhs[:, 1:2], bnb_sb[:], tv[:])
            smb = tsm.tile([128, 2], fp32, tag="tsm")
            nc.tensor.matmul(smb[:], bindT_r[:], brhs[:], start=True, stop=True)
            svec = tw.tile([128, 1], fp32, tag="tsvec")
            nc.vector.tensor_copy(svec[:], smb[:, 0:1])
            bvec = tw.tile([128, 1], fp32, tag="tbvec")
            nc.vector.tensor_copy(bvec[:], smb[:, 1:2])

            for u in range(nu):
                for cc in range(2):
                    cols = slice(cc * 2048, (cc + 1) * 2048)
                    nc.scalar.activation(F[:, u, cols], F[:, u, cols], Act.Relu,
                                         bias=bvec[:], scale=svec[:])
                nc.sync.dma_start(out[2 * u], F[0:64, u, :])
                nc.sync.dma_start(out[2 * u + 1], F[64:128, u, :])

    nc.compile()
    return nc


def make_in_maps(x, Wf, gnf_w, gnf_b, Wb, gnb_w, gnb_b, Wfu, bn_w, bn_b,
                 nsteps=T):
    B = x.shape[0]
    perm = np.concatenate([np.arange(0, 128), np.arange(192, 256),
                           np.arange(128, 192)])
    ind_m = np.zeros((128, 2), np.float32)
    ind_m[0:64, 0] = 1.0
    ind_m[64:128, 1] = 1.0
    indT_m = np.ascontiguousarray(ind_m.T)
    bind_m = np.zeros((128, 64), np.float32)
    for c in range(64):
        bind_m[c, c] = 1.0
        bind_m[c + 64, c] = 1.0
    bindT_m = np.ascontiguousarray(bind_m.T)
    Wfu2 = np.asarray(Wfu)[:, :, 0, 0]

    in_maps = []
    for core in range(N_CORES):
        b = core // 2
        fwd = core % 2 == 0
        xb = np.asarray(x)[b].reshape(-1, 64, S)[:nsteps]
        if not fwd:
            xb = xb[::-1]
        Wd = np.asarray(Wf if fwd else Wb)[perm]
        gw = np.asarray(gnf_w if fwd else gnb_w)[perm]
        gb = np.asarray(gnf_b if fwd else gnb_b)[perm]
        wconv_m = np.empty((9, 2, 128, 128), np.float32)
        for tap in range(9):
            dy, dx = tap // 3, tap % 3
            for half in range(2):
                wconv_m[tap, half] = Wd[half * 128:(half + 1) * 128, :, dy, dx].T
        wfu_m = np.zeros((128, 64), np.float32)
        wfu_m[64:128, :] = (Wfu2[:, 0:64] if fwd else Wfu2[:, 64:128]).T
        in_maps.append({
            "xs": np.ascontiguousarray(xb),
            "wconv": wconv_m,
            "wfu": wfu_m,
            "gnw": np.ascontiguousarray(gw.reshape(2, 128)),
            "gnb": np.ascontiguousarray(gb.reshape(2, 128)),
            "bnw": np.asarray(bn_w, np.float32).reshape(64, 1).copy(),
            "bnb": np.asarray(bn_b, np.float32).reshape(64, 1).copy(),
            "ind": ind_m,
            "indT": indT_m,
            "bind": bind_m,
            "bindT": bindT_m,
        })
    return in_maps


_cached_nc = None


def kernel(x, Wf, gnf_w, gnf_b, Wb, gnb_w, gnb_b, Wfu, bn_w, bn_b):
    global _cached_nc
    if _cached_nc is None:
        _cached_nc = build_program(T)
    nc = _cached_nc
    in_maps = make_in_maps(x, Wf, gnf_w, gnf_b, Wb, gnb_w, gnb_b, Wfu,
                           bn_w, bn_b)
    res = run_bass_kernel_spmd(nc, in_maps, CORE_IDS)
    outs = [res.results[2 * b]["out"].reshape(T, HID, 64, 64)
            for b in range(4)]
    return np.ascontiguousarray(np.stack(outs).astype(np.float32))

